# revision 31
# baseline (speedup 1.0000x reference)
"""Trainium2 Bass kernel: image -> additive-sinusoid audio encoding.

Math (per batch image b):
  gray = 255 * (w . rgb);  rev = flip(gray, rows);  avg = mean(gray)
  px   = clip(3*rev - 2*avg, 0, 255)
  A    = where(px==0, 0, exp(ln10 * (px/160 - 1.5)))            # [M=64 rows, N=64 cols]
  y[t] = sum_m A[m, col(t)] * sin(W[m]*t*dt + PHI0[m]),  col(t) = min(t//361, 63)
  audio= clip(0.5 + 2048*y, -32768, 32767)                       # [ns=23152]

Kernel strategy: t = n*361 + r  =>  angle = theta[i,n] + beta[i,r] (row flip folded
into the host tables), so  sinmat = sin(theta)cos(beta) + cos(theta)sin(beta).
P = A*sin(theta), Q = A*cos(theta) are fused into ONE K=128 matmul per image pair:
A is duplicated onto both partition halves with a tiny permutation matmul
(PE-dup; each matmul output gets its own psum tile - outputs at a free-offset
inside a tile fault on HW), then per pair
  y[(b2',n), r] = sum_{k<64} PQ[k]*cb[k,r] + sum_{k>=64} PQ[k]*sb[k-64,r]
with r running to 409 so the col-63 tail samples fall out of the same matmul.
Data-parallel over batch: 8 images per NeuronCore, layout
[128 partitions = (batch-half bh, image-row i), free = (b2, c, col)].
Host side: input is pre-permuted to the exact SBUF layout in fp16; output comes
back as fp16 y+0.5 and is clipped/cast during the unshard gather.
"""

import os

import numpy as np

# ---- problem constants (from the nn.Module definition; input-independent) ----
M = 64
N = 64
FL, FH, FS, T = 80.0, 7600.0, 22050, 1.05
NS = 2 * int(0.5 * FS * T)  # 23152
NUM = NS // N  # 361
RMAX = NS - (N - 1) * NUM  # 409 (last column's sample count)
NTAIL = RMAX - NUM  # 48
DT = float(np.float32(1.0 / FS))  # reference rounds dt to f32 (jnp weak typing)
TWO_PI = 2.0 * np.pi
B = 64
N_CORES = 8
B_LOC = B // N_CORES  # 8 images per core
SCALE_SSM = (0.5 / np.sqrt(M)) * 32768.0  # 2048
LN10 = float(np.log(10.0))
EXP_A = LN10 / 160.0
EXP_B = -1.5 * LN10
W0, W1, W2 = 0.2989, 0.5870, 0.1140
C00 = 3.0 * 255.0 * W0  # fold of the 3*255*w0 scale into the gray accumulator
R1 = W1 / W0
R2 = W2 / W0
KAVG2 = 2.0 * 255.0 * W0 / 4096.0  # sum(t) -> 2*avg(gray255) weighting

# single table: [BC 128 | T 128 | PERM 128 | CS 409 | pad 1]
TABW = 128 + 128 + 128 + RMAX + 1  # pad keeps row stride 4B-aligned for LDWEIGHTS
C_BC, C_T, C_PM, C_CS = 0, 128, 256, 384


def _make_tables():
    # LCG phase bank (faithful port, ir starts at 0)
    ia, ic, im = 9301, 49297, 233280
    ir = 0
    phi = []
    for _ in range(M):
        ir = (ir * ia + ic) % im
        phi.append(TWO_PI * ir / im)
    phi32 = np.array(phi, np.float64).astype(np.float32)
    w32 = (TWO_PI * FL * (FH / FL) ** (np.arange(M) / (M - 1))).astype(np.float32)

    # fold the row flip (tf.reverse on axis 1) into the tables: row i uses W[63-i]
    wf = w32[::-1].astype(np.float64)
    phif = phi32[::-1].astype(np.float64)

    n_idx = np.arange(N, dtype=np.float64)
    theta = wf[:, None] * (n_idx[None, :] * NUM * DT) + phif[:, None]  # [64, 64]
    stct = np.concatenate([np.sin(theta), np.cos(theta)], axis=0)  # [128, 64]
    # T[p, (b2', n)]: sin/cos(theta) broadcast over the 2 images of a pair
    Tt = np.tile(stct[:, None, :], (1, 2, 1)).reshape(128, 128)

    r_idx = np.arange(RMAX, dtype=np.float64)
    beta = wf[:, None] * (r_idx[None, :] * DT)  # [64, 409]
    cs = np.concatenate(
        [SCALE_SSM * np.cos(beta), SCALE_SSM * np.sin(beta)], axis=0
    )  # [128, 409]

    pp = np.arange(128)
    perm = (pp[:, None] % 64 == pp[None, :] % 64).astype(np.float64)  # [128,128]
    bc = KAVG2 * (pp[:, None] // 64 == pp[None, :] // 64).astype(np.float64)

    pad = np.zeros((128, 1))
    tabs = np.concatenate([bc, Tt, perm, cs, pad], axis=1).astype(np.float16)
    assert tabs.shape == (128, TABW), tabs.shape
    return {"tabs": np.ascontiguousarray(tabs)}


_TABLES = None


def tables():
    global _TABLES
    if _TABLES is None:
        _TABLES = _make_tables()
    return _TABLES


def build_nc():
    import concourse.bacc as bacc
    import concourse.bass as bass
    import concourse.mybir as mybir
    import concourse.tile as tile

    f32 = mybir.dt.float32
    f16 = mybir.dt.float16
    Alu = mybir.AluOpType
    Act = mybir.ActivationFunctionType

    nc = bacc.Bacc(
        "TRN2",
        target_bir_lowering=False,
        debug=False,
        num_devices=N_CORES,
        enable_asserts=False,
    )

    x16_d = nc.dram_tensor("x16", [128, 768], f16, kind="ExternalInput")
    tabs_d = nc.dram_tensor("tabs", [128, TABW], f16, kind="ExternalInput")
    audio_a_d = nc.dram_tensor("audio_a", [128, 2, RMAX], f16, kind="ExternalOutput")
    audio_b_d = nc.dram_tensor("audio_b", [128, 2, RMAX], f16, kind="ExternalOutput")

    with tile.TileContext(nc) as tc:
        with (
            tc.tile_pool(name="work", bufs=1) as work,
            tc.tile_pool(name="psum", bufs=1, space=bass.MemorySpace.PSUM) as psum,
        ):
            # Exp activation bias column (scalar engine requires an AP bias)
            expb = work.tile([128, 1], f32)
            nc.vector.memset(expb, float(EXP_B))

            # ---- input DMAs: X pre-permuted on host to the exact SBUF
            # layout [p=(bh,i), (b2, c, j)] with channel weights pre-folded;
            # X first then tables on one HWDGE ring ----
            X = work.tile([128, 768], f16)
            TB = work.tile([128, TABW], f16)
            nc.sync.dma_start(out=X, in_=x16_d[:])
            nc.sync.dma_start(out=TB, in_=tabs_d[:])
            Tt = TB[:, C_T : C_T + 128]
            CS = TB[:, C_CS : C_CS + RMAX]

            # ---- mean path first: per-image sums in ONE reduce (channels
            # pre-scaled on host, layout (b2, c, j)), f16 cast on GpSimd,
            # one block matmul reduces across partitions AND broadcasts ----
            rs = work.tile([128, 4], f32)
            nc.vector.reduce_sum(
                out=rs, in_=X.rearrange("p (b q) -> p b q", b=4),
                axis=mybir.AxisListType.X,
            )
            rs16 = work.tile([128, 4], f16)
            nc.gpsimd.tensor_scalar_mul(out=rs16, in0=rs, scalar1=1.0)
            csS2 = psum.tile([128, 4], f32)
            nc.tensor.matmul(
                csS2, TB[:, C_BC : C_BC + 128], rs16, start=True, stop=True
            )

            # ---- grayscale accumulate on DVE: t = sum_c x'_c ----
            Xb = X.rearrange("p (b c j) -> p b c j", b=4, c=3)
            t = work.tile([128, 4, 64], f32)
            nc.vector.tensor_add(out=t, in0=Xb[:, :, 1], in1=Xb[:, :, 0])
            nc.vector.tensor_add(out=t, in0=Xb[:, :, 2], in1=t)

            # ---- per image-pair s: px -> E -> A ----
            px = [work.tile([128, 2, 64], f32, name=f"px{s}") for s in range(2)]
            E = [work.tile([128, 128], f16, name=f"E{s}") for s in range(2)]
            A16 = [work.tile([128, 128], f16, name=f"A16_{s}") for s in range(2)]
            tv = t

            for s in range(2):
                nc.vector.scalar_tensor_tensor(
                    out=px[s], in0=tv[:, 2 * s : 2 * s + 2], scalar=float(C00),
                    in1=csS2[:, 2 * s : 2 * s + 2].broadcast_to([128, 2, 64]),
                    op0=Alu.mult, op1=Alu.subtract,
                )
                nc.vector.tensor_scalar(
                    out=px[s], in0=px[s], scalar1=0.0, scalar2=255.0,
                    op0=Alu.max, op1=Alu.min,
                )
                nc.scalar.activation(
                    out=E[s], in_=px[s].rearrange("p a b -> p (a b)"),
                    func=Act.Exp, bias=expb, scale=float(EXP_A),
                )

            # ---- A masks + PE duplication (all four halves up front) ----
            a2 = {}
            for s in range(2):
                nc.vector.scalar_tensor_tensor(
                    out=A16[s], in0=px[s].rearrange("p a b -> p (a b)"),
                    scalar=0.0, in1=E[s], op0=Alu.is_gt, op1=Alu.mult,
                )
                a2[s, 0] = psum.tile([128, 128], f32, name=f"a2lo{s}")
                a2[s, 1] = psum.tile([128, 128], f32, name=f"a2hi{s}")
                nc.tensor.matmul(
                    a2[s, 0], TB[0:64, C_PM : C_PM + 128], A16[s][0:64],
                    start=True, stop=True,
                )
                nc.tensor.matmul(
                    a2[s, 1], TB[64:128, C_PM : C_PM + 128], A16[s][64:128],
                    start=True, stop=True,
                )

            # ---- PQ = A2 * T, then one K=128 N=409 matmul per image pair ----
            PQ = [
                work.tile([128, 128], f16, name=f"PQ{s}{h}")
                for s in range(2) for h in range(2)
            ]
            y = {}
            for g in (0, 2, 1, 3):  # (bh, s) completion order
                bh, s = g // 2, g % 2
                pq = PQ[2 * s + bh]
                nc.vector.tensor_mul(out=pq, in0=a2[s, bh], in1=Tt)
                ytag = {0: "ya", 2: "yb", 1: "yc", 3: "ya"}[g]
                yt = psum.tile([128, RMAX], f32, tag=ytag, bufs=1, name=f"y{g}")
                nc.tensor.matmul(yt, pq, CS, start=True, stop=True)
                y[g] = yt

            # ---- PSUM drain: u = y + 0.5, fp16; clip happens on host where
            # clip(0.5+v,lo,hi) == 0.5+clip(v,lo-.5,hi-.5) (f16 inf clips too) ----
            Ua = work.tile([128, 2, RMAX], f16)
            Ub = work.tile([128, 2, RMAX], f16)
            slot = {0: (Ua, 0), 2: (Ua, 1), 1: (Ub, 0), 3: (Ub, 1)}
            HCUT = 208
            for g in (0, 2, 1, 3):
                U, k = slot[g]
                nc.vector.tensor_scalar(
                    out=U[:, k, 0:HCUT], in0=y[g][:, 0:HCUT],
                    scalar1=0.5, scalar2=0.0, op0=Alu.add, op1=Alu.bypass,
                )
                nc.scalar.activation(
                    out=U[:, k, HCUT:RMAX], in_=y[g][:, HCUT:RMAX],
                    func=Act.Copy, bias=0.5, scale=1.0,
                )
            nc.sync.dma_start(out=audio_a_d[:], in_=Ua)
            nc.scalar.dma_start(out=audio_b_d[:], in_=Ub)

    nc.compile()
    return nc


_NC = None


def _get_nc():
    global _NC
    if _NC is None:
        _NC = build_nc()
    return _NC


LAST_RESULTS = None


def kernel(x: np.ndarray) -> np.ndarray:
    from concourse.bass_utils import run_bass_kernel_spmd

    x = np.asarray(x, dtype=np.float32)
    assert x.shape == (B, 64, 64, 3), x.shape

    # shard + permute to the SBUF layout [p=(bh,i), (b2, c, j)], fp16,
    # with the grayscale channel weights pre-folded (c scaled by w_c/w0)
    xc = x.reshape(N_CORES, 2, 4, 64, 64, 3)  # [core, bh, b2, i, j, c]
    xs = xc * np.array([1.0, R1, R2], np.float32)
    x16 = xs.transpose(0, 1, 3, 2, 5, 4).reshape(N_CORES, 128, 768)
    x16 = np.ascontiguousarray(x16).astype(np.float16)

    nc = _get_nc()
    tbl = tables()
    in_maps = []
    for c in range(N_CORES):
        m = {"x16": x16[c]}
        m.update(tbl)
        in_maps.append(m)

    trace = os.environ.get("BASS_KERNEL_TRACE", "0") == "1"
    res = run_bass_kernel_spmd(
        nc, in_maps, core_ids=list(range(N_CORES)), trace=trace
    )
    global LAST_RESULTS
    LAST_RESULTS = res

    outs = np.empty((B, NS), np.float32)
    for c, r in enumerate(res.results):
        # audio_a[p=(b2',n), {g0,g2}, r]; audio_b[p, {g1,g3}, r]
        aa = r["audio_a"].astype(np.float32).reshape(2, 64, 2, RMAX)
        bb = r["audio_b"].astype(np.float32).reshape(2, 64, 2, RMAX)
        for b_loc in range(B_LOC):
            bh, s, b2p = b_loc // 4, (b_loc // 2) % 2, b_loc % 2
            arr, k = (aa, bh) if s == 0 else (bb, bh)
            img = arr[b2p, :, k]  # [64, RMAX]
            row = c * B_LOC + b_loc
            outs[row, : N * NUM] = img[:, :NUM].reshape(N * NUM)
            outs[row, N * NUM :] = img[63, NUM:]
    np.clip(outs, -32768.0, 32767.0, out=outs)
    return outs
